# revision 39
# baseline (speedup 1.0000x reference)
"""DCT-compressed attention (nn_DCTAttentionIdeal) on 8 Trainium2 NeuronCores.

Math (per head, reference ordering):
    S    = (Q*s) @ (K*mask*s)^T with s = D**-0.25             [N,N]
    atn  = softmax(S, axis=-1)
    Vd   = Qd @ (V*mask)                                      [M,D]
    out  = Qd^T @ ((Qd @ atn @ Qd^T) @ Vd)                    [N,D]

Kernel reshaping (exact in real arithmetic):
  - softmax max-subtraction replaced by a global bias: exp(S/8 - 3.2).
    The bias cancels in the normalization; it keeps exp outputs < 240 so
    they are representable in fp8e4 (TRN fp8 overflows to Inf, max 240).
  - per-row 1/denom folded into DCT columns:
        A1^T[k,m] = sum_q exp(S)[q,k] * (C*Qd^T[q,m]/denom[q])
    so the [N,N] exp matrix is consumed unnormalized straight from SBUF.
    C = 2**16 positions cq in fp8's normal range; it is divided back out
    of the host-side Qd used by the final projection (Qd / C^2: one C from
    cq, one from the C-scaled Qd used in Vd).
  - final contraction reassociated: out = Qd^T @ (G @ Vd), G = A1 @ Qd^T.

dtypes: the O(N^2 M) matmul (exp -> A1^T) runs fp8e4 with
perf_mode=DoubleRow: each matmul contracts TWO 128-q-blocks at once
(lhsT [128,2,128] exp pair, rhs [128,2,256] cq pair) for ~2x bf16
throughput (measured ~110ns per 2-block 256-wide matmul).  All other
stationary operands are bf16 (enables fast weight load); moving
operands bf16.  Accumulation is always f32.

Pipeline (per core, 8 slots of 8 q-blocks):
  - A-phase of slot i+2 (score matmuls, exp on ScalarE, 1/denom+cq on
    DVE) is interleaved into the B-phase (A1^T DoubleRow matmuls) of
    slot i: 2-slot lookahead, since ScalarE (~179us of exp+accum-read)
    is the pacing engine.
  - a_fin (DVE reduce/recip/cq) trails its exp ACTs by 4 b-steps so a
    not-yet-finished ACT never head-of-line-blocks DVE behind the a1
    PSUM->SBUF drain casts the PE is waiting on.
  - A1 PSUM drains are kc-paired ([128,2,256] in one bank, one strided
    cast) halving cast count and PSUM WAR syncs.
  - the final out-projection of head h is deferred into the next
    slot's b-steps (2 q-blocks per step) so its PSUM-bank ping-pong
    hides behind DoubleRow matmuls.
PSUM (8 banks) is fully committed: scores 2x[128,1024] (4), a1
2x[128,2,256] (2), GT accum [128,512] (1), misc [128,64] (1).

Sharding: batch*heads (2*16=32) split 4-per-core across 8 cores; Q_dct
replicated; no cross-core communication.  Host pre-transposes Q/K and
Q_dct and pre-scales Q_dct copies (pure layout/constant scaling);
masking, softmax and all DCT algebra run on-device.
"""

import numpy as np
import ml_dtypes

import concourse.tile as tile
from concourse import bacc, mybir
from concourse import bass_utils

F32 = mybir.dt.float32
BF16 = mybir.dt.bfloat16
F32R = mybir.dt.float32r
FP8 = mybir.dt.float8e4
NPBF16 = ml_dtypes.bfloat16
AF = mybir.ActivationFunctionType
ALU = mybir.AluOpType
AX = mybir.AxisListType
DR = mybir.MatmulPerfMode.DoubleRow

B, H, N, D, M = 2, 16, 2048, 64, 256
NCORES = 8
HPC = (B * H) // NCORES  # heads per core = 4
NT = N // 128            # 16 (q and k 128-blocks)
MT = M // 128             # 2
NQG = 2                  # q-group count (software pipeline A||B)

CSCALE = 2.0 ** 16       # cq fp8 positioning scale (folded back via QdNr)
EXP_BIAS = -3.2          # keeps exp(S/8+bias) < 240 (fp8e4 max; Inf above)


def _emit(tc, ctx, io):
    nc = tc.nc
    P = 128
    GQ = NT // NQG               # q-blocks per group = 8
    SCH = min(1024, N)           # score chunk (elements) per activation
    NCH = N // SCH               # activations per q-block

    sh = ctx.enter_context(tc.tile_pool(name="shared", bufs=1))
    exp_pool = ctx.enter_context(tc.tile_pool(name="exp", bufs=12))  # 3 groups x 4 pairs
    kt_pool = ctx.enter_context(tc.tile_pool(name="ktr", bufs=3))
    qt_pool = ctx.enter_context(tc.tile_pool(name="qtr", bufs=3))
    vm_pool = ctx.enter_context(tc.tile_pool(name="vmask", bufs=3))
    cq_pool = ctx.enter_context(tc.tile_pool(name="cq", bufs=2))
    a1_pool = ctx.enter_context(tc.tile_pool(name="a1t", bufs=2))
    gt_pool = ctx.enter_context(tc.tile_pool(name="gt", bufs=2))
    vd_pool = ctx.enter_context(tc.tile_pool(name="vd", bufs=2))
    y_pool = ctx.enter_context(tc.tile_pool(name="y", bufs=2))
    ost_pool = ctx.enter_context(tc.tile_pool(name="ost", bufs=2))
    msk_pool = ctx.enter_context(tc.tile_pool(name="msk", bufs=3))
    st_pool = ctx.enter_context(tc.tile_pool(name="stats", bufs=8))

    ps_s = ctx.enter_context(tc.tile_pool(name="ps_s", bufs=2, space="PSUM"))
    # 8 PSUM banks: scores 2x2, a1 3x1, one shared bank ("g") for every
    # small accumulator -- its users (Vd, GT, yt, out-projections) are
    # time-disjoint across slot phases, which frees a bank to deepen the
    # a1 drain ring to 6 k-blocks (the PE's top WAR stall).
    ps_a1 = ctx.enter_context(tc.tile_pool(name="ps_a1", bufs=3, space="PSUM"))
    ps_gt = ctx.enter_context(tc.tile_pool(name="ps_gt", bufs=1, space="PSUM"))

    # --- shared, once per core (declared here, DMA'd in the prologue in
    # priority order: head-0 inputs first, tail-only operands last) -------
    maskB = sh.tile([64, N], F32)       # mask row broadcast over d-partitions
    qdtr = sh.tile([P, NT, M], BF16)    # C * Qd^T (Vd lhsT + cq source)
    qdt16 = sh.tile([P, NT, M], BF16)   # Qd^T (GT lhsT)
    qdnr = sh.tile([P, MT, N], BF16)    # Qd/C^2 [m,q] (out lhsT)
    ebias = sh.tile([P, 1], F32)        # exp bias (per-partition const AP)

    def shared_dma_early():
        nc.sync.dma_start(maskB[:], io["maskB"])
        nc.sync.dma_start(qdtr[:], io["QdTc"].rearrange("(t p) m -> p t m", p=P))
        nc.vector.memset(ebias[:], EXP_BIAS)

    def shared_dma_late():
        nc.sync.dma_start(qdt16[:], io["QdT16"].rearrange("(t p) m -> p t m", p=P))
        nc.sync.dma_start(qdnr[:], io["QdNc"].rearrange("(c p) q -> p c q", p=P))

    state = [None] * HPC

    def prep_dma(h):
        st = state[h] = {}
        st["qt"] = qt_pool.tile([64, N], BF16, name="qt", tag="qt")
        nc.sync.dma_start(st["qt"][:], io["QT"][h])
        st["kts"] = kt_pool.tile([64, N], BF16, name="kts", tag="kts")
        nc.sync.dma_start(st["kts"][:], io["KT"][h])
        st["mk"] = msk_pool.tile([P, NT], F32, name="mk", tag="mk")
        nc.sync.dma_start(st["mk"][:], io["maskT"][h])
        st["vm"] = vm_pool.tile([P, NT, D], BF16, name="vm", tag="vm")
        nc.sync.dma_start(st["vm"][:], io["V"][h].rearrange("(t p) d -> p t d", p=P))

    def prep_mask(h):
        # masking runs on GpSimd (idle engine) to keep DVE's queue short;
        # it is emitted >=1 slot before any consumer so latency is hidden.
        st = state[h]
        vm, mk, kts = st["vm"], st["mk"], st["kts"]
        nc.vector.tensor_mul(kts[:], kts[:], maskB[:])
        for t in range(NT):
            nc.vector.tensor_scalar_mul(vm[:, t, :], vm[:, t, :], mk[:, t : t + 1])
        st["cq"] = cq_pool.tile([P, NT, M], FP8, name="cq", tag="cq")
        st["a1"] = a1_pool.tile([P, NT, NQG, M], BF16, name="a1", tag="a1")
        st["exps"] = {}
        st["sums"] = {}

    def prep_vd(h):
        # Vd = (C*Qd) @ (V*m) -> [M, D]; emitted a slot after prep_mask so
        # the PE never queues behind the masking.
        st = state[h]
        vm = st["vm"]
        vd = st["vd"] = vd_pool.tile([P, MT, D], BF16, name="vd", tag="vd")
        for mh in range(MT):
            vps = ps_gt.tile([P, D], F32, name="vps", tag="g")
            for t in range(NT):
                nc.tensor.matmul(
                    vps[:],
                    lhsT=qdtr[:, t, mh * P : (mh + 1) * P],
                    rhs=vm[:, t, :],
                    start=(t == 0),
                    stop=(t == NT - 1),
                )
            nc.vector.tensor_copy(vd[:, mh, :], vps[:])

    def a_chunk(h, q, c):
        st = state[h]
        pr, par = q // 2, q % 2
        if par == 0 and c == 0:
            st["exps"][pr] = exp_pool.tile([P, 2, N], FP8, name="exp", tag="exp")
        ex = st["exps"][pr]
        if c == 0:
            st["sums"][q] = st_pool.tile([P, NCH], F32, name="sums", tag="sums")
        sums = st["sums"][q]
        sps = ps_s.tile([P, SCH], F32, name="s", tag="s")
        for j in range(SCH // 512):
            nc.tensor.matmul(
                sps[:, j * 512 : (j + 1) * 512],
                lhsT=st["qt"][:, q * P : (q + 1) * P],
                rhs=st["kts"][:, c * SCH + j * 512 : c * SCH + (j + 1) * 512],
                start=True,
                stop=True,
            )
        nc.scalar.activation(
            ex[:, par, c * SCH : (c + 1) * SCH],
            sps[:],
            AF.Exp,
            scale=0.125,
            bias=ebias[:],
            accum_out=sums[:, c : c + 1],
        )

    def a_fin(h, q):
        st = state[h]
        sums = st["sums"].pop(q)
        den = st_pool.tile([P, 1], F32, name="den", tag="den")
        if NCH > 1:
            nc.vector.tensor_reduce(den[:], sums[:], axis=AX.X, op=ALU.add)
        else:
            den = sums
        rec = st_pool.tile([P, 1], F32, name="rec", tag="rec")
        nc.vector.reciprocal(rec[:], den[:])
        nc.vector.tensor_scalar_mul(st["cq"][:, q, :], qdtr[:, q, :], rec[:])

    def b_kc2(h, g, j):
        # A1^T for k-blocks 2j, 2j+1: two accumulation chains into one
        # PSUM bank, drained by a single strided cast (halves DVE casts
        # and the PSUM WAR sync count).
        st = state[h]
        aps_ = ps_a1.tile([P, 2, M], F32, name="a1", tag="a1")
        NPR = GQ // 2  # DoubleRow pairs per group = 4
        for half in range(2):
            kc = 2 * j + half
            for pi in range(NPR):
                pr = (g * GQ) // 2 + pi
                nc.tensor.matmul(
                    aps_[:, half, :],
                    lhsT=st["exps"][pr][:, :, kc * P : (kc + 1) * P],
                    rhs=st["cq"][:, g * GQ + 2 * pi : g * GQ + 2 * pi + 2, :],
                    start=(pi == 0),
                    stop=(pi == NPR - 1),
                    perf_mode=DR,
                )
        nc.vector.tensor_copy(st["a1"][:, 2 * j : 2 * j + 2, g, :], aps_[:])

    def tail_gtyt(h):
        st = state[h]
        gt = gt_pool.tile([P, MT, M], BF16, name="gt", tag="gt")
        gps = ps_gt.tile([P, MT * M], F32, name="g", tag="g")
        for nh in range(MT):
            for kc in range(NT):
                for g in range(NQG):
                    nc.tensor.matmul(
                        gps[:, nh * M : (nh + 1) * M],
                        lhsT=qdt16[:, kc, nh * P : (nh + 1) * P],
                        rhs=st["a1"][:, kc, g, :],
                        start=(kc == 0 and g == 0),
                        stop=(kc == NT - 1 and g == NQG - 1),
                    )
            nc.vector.tensor_copy(gt[:, nh, :], gps[:, nh * M : (nh + 1) * M])

        yt = st["yt"] = y_pool.tile([P, MT, D], BF16, name="yt", tag="yt")
        for mh in range(MT):
            yps = ps_gt.tile([P, D], F32, name="yps", tag="g")
            for nh in range(MT):
                nc.tensor.matmul(
                    yps[:],
                    lhsT=gt[:, nh, mh * P : (mh + 1) * P],
                    rhs=st["vd"][:, nh, :],
                    start=(nh == 0),
                    stop=(nh == MT - 1),
                )
            nc.vector.tensor_copy(yt[:, mh, :], yps[:])

    def out_q2(h, u):
        # final projection for q-blocks 2u, 2u+1; interleaved into the
        # NEXT slot's b-steps so the copy/DMA latency hides behind DR
        # matmuls.  Two accumulation chains share one PSUM tile and are
        # drained by a single copy + DMA (halves DVE ops and WAR syncs).
        st = state[h]
        yt = st["yt"]
        o_r = io["out"][h].rearrange("(u w p) d -> u p w d", w=2, p=P)
        ops_ = ps_gt.tile([P, 2, D], F32, name="ops", tag="g")
        for w in range(2):
            q = 2 * u + w
            for mh in range(MT):
                nc.tensor.matmul(
                    ops_[:, w, :],
                    lhsT=qdnr[:, mh, q * P : (q + 1) * P],
                    rhs=yt[:, mh, :],
                    start=(mh == 0),
                    stop=(mh == MT - 1),
                )
        ost = ost_pool.tile([P, 2, D], F32, name="ost", tag="ost")
        nc.vector.tensor_copy(ost[:], ops_[:])
        nc.sync.dma_start(o_r[u], ost[:])

    # --- software-pipelined emission over group slots -------------------
    # A-phase (scores+exp+cq) of slot i+2 is interleaved into the B-phase
    # (A1^T DoubleRow matmuls) of slot i: a 2-slot lookahead so ScalarE's
    # exp stream runs a full slot ahead of the PE's consumption of cq/exp.
    slots = [(h, g) for h in range(HPC) for g in range(NQG)]
    prep_dma(0)
    shared_dma_early()
    if HPC > 1:
        prep_dma(1)
    shared_dma_late()
    prep_mask(0)
    prep_vd(0)
    for q in range(2 * GQ):  # A-phase for slots 0 and 1 (head 0, both groups)
        for c in range(NCH):
            a_chunk(0, q, c)
        a_fin(0, q)

    afin_queue = []   # a_fin delayed 2 b-steps so its DVE chain never
    #                   stalls the queue on a not-yet-finished ACT
    pending_out = []  # (h, q) out-projections deferred into the next slot

    def drain_afin(keep):
        while len(afin_queue) > keep:
            afin_queue.pop(0)()

    for i, (h, g) in enumerate(slots):
        if i % 2 == 0:
            h_dma = (i + 4) // 2
            if h_dma < HPC:
                prep_dma(h_dma)
            h_msk = (i + 2) // 2
            if h_msk < HPC:
                prep_mask(h_msk)
        else:
            h_vd = (i + 1) // 2
            if h_vd < HPC:
                prep_vd(h_vd)
        tgt = i + 2
        for j in range(NT // 2):
            b_kc2(h, g, j)
            if tgt < len(slots):
                th_, tg_ = slots[tgt]
                q = tg_ * GQ + j
                for c in range(NCH):
                    a_chunk(th_, q, c)
                afin_queue.append(lambda th_=th_, q=q: a_fin(th_, q))
                drain_afin(4)
            else:
                drain_afin(0)
            if pending_out:
                ph, pu = pending_out.pop(0)
                out_q2(ph, pu)
        if g == NQG - 1:
            tail_gtyt(h)
            pending_out.extend((h, u) for u in range(NT // 2))
    drain_afin(0)
    for ph, pu in pending_out:
        out_q2(ph, pu)


def build_nc():
    from contextlib import ExitStack

    nc = bacc.Bacc("TRN2", target_bir_lowering=False, debug=False)
    io = {
        "QT": nc.dram_tensor("QT", [HPC, 64, N], BF16, kind="ExternalInput").ap(),
        "KT": nc.dram_tensor("KT", [HPC, 64, N], BF16, kind="ExternalInput").ap(),
        "V": nc.dram_tensor("V", [HPC, N, D], BF16, kind="ExternalInput").ap(),
        "maskT": nc.dram_tensor("maskT", [HPC, 128, NT], F32, kind="ExternalInput").ap(),
        "maskB": nc.dram_tensor("maskB", [64, N], F32, kind="ExternalInput").ap(),
        "QdTc": nc.dram_tensor("QdTc", [N, M], BF16, kind="ExternalInput").ap(),
        "QdT16": nc.dram_tensor("QdT16", [N, M], BF16, kind="ExternalInput").ap(),
        "QdNc": nc.dram_tensor("QdNc", [M, N], BF16, kind="ExternalInput").ap(),
        "out": nc.dram_tensor("out", [HPC, N, D], F32, kind="ExternalOutput").ap(),
    }
    with tile.TileContext(nc) as tc:
        with ExitStack() as ctx:
            _emit(tc, ctx, io)
    nc.compile()
    return nc


_NC = None


def _get_nc():
    global _NC
    if _NC is None:
        _NC = build_nc()
    return _NC


def make_in_maps(Q, K, V, mask, Q_dct):
    Q = np.asarray(Q, dtype=np.float32).reshape(B * H, N, D)
    K = np.asarray(K, dtype=np.float32).reshape(B * H, N, D)
    V = np.asarray(V, dtype=np.float32).reshape(B * H, N, D)
    mask = np.asarray(mask, dtype=np.float32)
    Q_dct = np.asarray(Q_dct, dtype=np.float32)

    QT = np.ascontiguousarray(Q.transpose(0, 2, 1)).astype(NPBF16)
    KT = np.ascontiguousarray(K.transpose(0, 2, 1)).astype(NPBF16)
    V16 = V.astype(NPBF16)
    QdT = np.ascontiguousarray(Q_dct.T)
    QdTc = (QdT * CSCALE).astype(NPBF16)
    QdT16 = QdT.astype(NPBF16)
    QdNc = (np.ascontiguousarray(Q_dct) / (CSCALE * CSCALE)).astype(NPBF16)
    # maskT[b, p, t] = mask[b, t*128 + p]
    maskT = np.ascontiguousarray(mask.reshape(B, NT, 128).transpose(0, 2, 1))

    in_maps = []
    for c in range(NCORES):
        sl = slice(HPC * c, HPC * (c + 1))
        heads = range(HPC * c, HPC * (c + 1))
        in_maps.append(
            {
                "QT": np.ascontiguousarray(QT[sl]),
                "KT": np.ascontiguousarray(KT[sl]),
                "V": np.ascontiguousarray(V16[sl]),
                "maskT": np.ascontiguousarray(
                    np.stack([maskT[hp // H] for hp in heads])
                ),
                "maskB": np.ascontiguousarray(
                    np.broadcast_to(mask[(HPC * c) // H][None, :], (64, N))
                ),
                "QdTc": QdTc,
                "QdT16": QdT16,
                "QdNc": QdNc,
            }
        )
    return in_maps


def run_on_device(in_maps, **kwargs):
    nc = _get_nc()
    return bass_utils.run_bass_kernel_spmd(
        nc, in_maps, core_ids=list(range(NCORES)), **kwargs
    )


def kernel(Q, K, V, mask, Q_dct):
    in_maps = make_in_maps(Q, K, V, mask, Q_dct)
    res = run_on_device(in_maps)
    out = np.empty((B * H, N, D), dtype=np.float32)
    for c in range(NCORES):
        out[HPC * c : HPC * (c + 1)] = res.results[c]["out"]
    return out.reshape(B, H, N, D)
